# revision 1
# baseline (speedup 1.0000x reference)
"""DGCNN (4x GCNConv + SortPool + Conv1d head) on 8 Trainium2 NeuronCores.

Data-parallel over graphs: each core owns 64 of the 512 graphs.
Per graph the GCN aggregation is computed densely:
    agg^T[f, i] = sum_j (y[j, f] * dinv[j]) * (adj[j, i] * dinv[i])
with adj the src-major dense adjacency-with-self-loops count matrix,
densified on the host from edge_index (a re-layout of the integer graph
structure); all floating-point math (degrees, rsqrt, normalization, 4 GCN
layers, SortPool top-k selection+gather, conv/MLP head) runs on-device.
"""

import numpy as np

B = 512
M = 200
GPC = 64            # graphs per core
NPC = GPC * M       # nodes per core
NCORES = 8
K = 30
F = 97

_STATE = {}


def _apf(base, pairs):
    """AP with the partition dim of `base` and custom free [step,count] pairs."""
    import concourse.bass as bass
    return bass.AP(tensor=base.tensor, offset=base.offset,
                   ap=[list(base.ap[0])] + [list(p) for p in pairs])


def _build(skip=()):
    skip = set(skip)
    import concourse.bass as bass
    import concourse.bacc as bacc
    import concourse.mybir as mybir
    from concourse.tile import TileContext
    from concourse.masks import make_identity

    fp32 = mybir.dt.float32
    AF = mybir.ActivationFunctionType
    OP = mybir.AluOpType

    nc = bacc.Bacc("TRN2", target_bir_lowering=False, debug=False,
                   num_devices=NCORES)

    x_d = nc.dram_tensor("x", [NPC, 128], fp32, kind="ExternalInput")
    adj_d = nc.dram_tensor("adj", [GPC * M, M], fp32, kind="ExternalInput")
    w1_d = nc.dram_tensor("w1", [128, 32], fp32, kind="ExternalInput")
    w234_d = nc.dram_tensor("w234", [96, 32], fp32, kind="ExternalInput")
    bgcn_d = nc.dram_tensor("bgcn", [32, 4], fp32, kind="ExternalInput")
    cw1_d = nc.dram_tensor("cw1", [97, 16], fp32, kind="ExternalInput")
    cb1_d = nc.dram_tensor("cb1", [16, 1], fp32, kind="ExternalInput")
    cw2_d = nc.dram_tensor("cw2", [80, 32], fp32, kind="ExternalInput")
    cb2_d = nc.dram_tensor("cb2", [32, 1], fp32, kind="ExternalInput")
    lw1_d = nc.dram_tensor("lw1", [352, 128], fp32, kind="ExternalInput")
    lb1_d = nc.dram_tensor("lb1", [128, 1], fp32, kind="ExternalInput")
    lw2_d = nc.dram_tensor("lw2", [128, 1], fp32, kind="ExternalInput")
    lb2_d = nc.dram_tensor("lb2", [1, 1], fp32, kind="ExternalInput")

    h4buf_d = nc.dram_tensor("h4buf", [GPC, 256], fp32, kind="Internal")
    idxbuf_d = nc.dram_tensor("idxbuf", [GPC * 32], mybir.dt.int16,
                              kind="Internal")
    out_d = nc.dram_tensor("out", [1, GPC], fp32, kind="ExternalOutput")

    with TileContext(nc) as tc:
        with tc.tile_pool(name="const", bufs=1) as cp:
            ident = cp.tile([128, 128], fp32)
            make_identity(nc, ident[:])
            ones = cp.tile([128, 128], fp32)
            nc.vector.memset(ones[:], 1.0)
            w1 = cp.tile([128, 32], fp32)
            nc.sync.dma_start(w1[:], w1_d.ap())
            # rows 0:32 = W2, 32:64 = W3, 64:96 = W4 (padded to 32 cols)
            w234 = cp.tile([96, 32], fp32)
            nc.sync.dma_start(w234[:], w234_d.ap())
            bgcn = cp.tile([32, 4], fp32)
            nc.sync.dma_start(bgcn[:], bgcn_d.ap())
            cw1 = cp.tile([97, 16], fp32)
            nc.sync.dma_start(cw1[:], cw1_d.ap())
            cb1 = cp.tile([16, 1], fp32)
            nc.sync.dma_start(cb1[:], cb1_d.ap())
            cw2 = [cp.tile([16, 32], fp32, tag=f"cw2_{t}", name=f"cw2_{t}")
                   for t in range(5)]
            for t in range(5):
                nc.sync.dma_start(cw2[t][:], cw2_d.ap()[16 * t:16 * t + 16, :])
            cb2 = cp.tile([32, 1], fp32)
            nc.sync.dma_start(cb2[:], cb2_d.ap())
            lw1 = [cp.tile([128, 128], fp32, tag=f"lw1_{q}", name=f"lw1_{q}")
                   for q in range(3)]
            nc.sync.dma_start(lw1[0][:], lw1_d.ap()[0:128, :])
            nc.sync.dma_start(lw1[1][:], lw1_d.ap()[128:256, :])
            nc.sync.dma_start(lw1[2][0:96, :], lw1_d.ap()[256:352, :])
            lb1 = cp.tile([128, 1], fp32)
            nc.sync.dma_start(lb1[:], lb1_d.ap())
            lw2 = cp.tile([128, 1], fp32)
            nc.sync.dma_start(lw2[:], lw2_d.ap())
            lb2 = cp.tile([1, 1], fp32)
            nc.sync.dma_start(lb2[:], lb2_d.ap())

            # hcat rows: 0:32 h1, 32:64 h2, 64:96 h3, 96 h4; 112 partitions
            # (multiple of 16 for ap_gather); col = 256*g + i.
            hcat = cp.tile([112, 256 * GPC], fp32)
            topsT = cp.tile([112, 32 * GPC], fp32)
            # x transposed to [feature, node] once, col = global node id
            xT = cp.tile([128, NPC], fp32)
            if "agg" in skip:
                nc.gpsimd.memset(hcat[:], 0.25)

            with (
                tc.tile_pool(name="work", bufs=4) as wp,
                tc.tile_pool(name="adjp", bufs=5) as ap_pool,
                tc.tile_pool(name="psA", bufs=2, space="PSUM") as psA,
                tc.tile_pool(name="psY", bufs=3, space="PSUM") as psY,
                tc.tile_pool(name="psG", bufs=3, space="PSUM") as psG,
            ):
                # ---- x load (4 big DMAs) + PE transpose to xT ----
                if "xT" not in skip:
                    for q in range(4):
                        xs = wp.tile([128, 3200], fp32, tag="xs", bufs=2,
                                     name="xs")
                        if "dma_x" not in skip:
                            nc.gpsimd.dma_start(
                                xs[:],
                                x_d.ap()[3200 * q:3200 * (q + 1), :]
                                .rearrange("(c p) f -> p c f", p=128))
                        for c in range(25):
                            pxt = psA.tile([128, 224], fp32, tag="deg",
                                           name="pxt")
                            nc.tensor.transpose(
                                pxt[:, 0:128], xs[:, 128 * c:128 * (c + 1)],
                                ident[:])
                            nc.vector.tensor_copy(
                                xT[:, 3200 * q + 128 * c:
                                   3200 * q + 128 * (c + 1)],
                                pxt[:, 0:128])
                else:
                    nc.gpsimd.memset(xT[:], 1.0)

                for gp in range(GPC // 2):
                    pair = []
                    dinv = wp.tile([128, 404], fp32, tag="dinv", name="dinv")
                    rec = wp.tile([128, 404], fp32, tag="rec", name="rec")
                    for half in range(2):
                        g = 2 * gp + half
                        do = 202 * half
                        # ---- adjacency load + degrees + normalization ----
                        adj_lo = ap_pool.tile([128, 200], fp32, tag="adj_lo",
                                              name="adj_lo")
                        adj_hi = ap_pool.tile([72, 200], fp32, tag="adj_hi",
                                              name="adj_hi")
                        if "dma_adj" not in skip:
                            nc.sync.dma_start(
                                adj_lo[:], adj_d.ap()[200 * g:200 * g + 128, :])
                            nc.scalar.dma_start(
                                adj_hi[:],
                                adj_d.ap()[200 * g + 128:200 * g + 200, :])
                        pdeg = psA.tile([128, 224], fp32, tag="deg",
                                        name="pdeg")
                        if "deg128" not in skip:
                            # deg replicated on 128 partitions: ones^T @ adj
                            nc.tensor.matmul(pdeg[:, 0:200], lhsT=ones[:],
                                             rhs=adj_lo[:],
                                             start=True, stop=False)
                            nc.tensor.matmul(pdeg[:, 0:200],
                                             lhsT=ones[0:72, :],
                                             rhs=adj_hi[:],
                                             start=False, stop=True)
                        if "degcol" not in skip:
                            # deg as a column (node j on partition j): adj @ 1
                            nc.tensor.matmul(pdeg[0:128, 200:201],
                                             lhsT=adj_lo[:, 0:128],
                                             rhs=ones[:, 0:1],
                                             start=True, stop=False)
                            nc.tensor.matmul(pdeg[0:128, 200:201],
                                             lhsT=adj_hi[:, 0:128],
                                             rhs=ones[0:72, 0:1],
                                             start=False, stop=True)
                            nc.tensor.matmul(pdeg[0:72, 201:202],
                                             lhsT=adj_lo[:, 128:200],
                                             rhs=ones[:, 0:1],
                                             start=True, stop=False)
                            nc.tensor.matmul(pdeg[0:72, 201:202],
                                             lhsT=adj_hi[:, 128:200],
                                             rhs=ones[0:72, 0:1],
                                             start=False, stop=True)
                        if "dinv" not in skip:
                            nc.vector.reciprocal(rec[:, do:do + 202],
                                                 pdeg[:, 0:202])
                            nc.scalar.activation(dinv[:, do:do + 202],
                                                 rec[:, do:do + 202], AF.Sqrt)
                        elif half == 0:
                            nc.gpsimd.memset(dinv[:], 1.0)
                        # adjS = adj[j,i] * dinv[i]  (col scale; the row scale
                        # dinv[j] is folded into the y drain below)
                        adjS_lo = ap_pool.tile([128, 200], fp32, tag="adjS_lo",
                                               name="adjS_lo")
                        adjS_hi = ap_pool.tile([72, 200], fp32, tag="adjS_hi",
                                               name="adjS_hi")
                        if "adjS" not in skip:
                            nc.gpsimd.tensor_tensor(
                                out=adjS_lo[:], in0=adj_lo[:],
                                in1=dinv[:, do:do + 200], op=OP.mult)
                            nc.gpsimd.tensor_tensor(
                                out=adjS_hi[:], in0=adj_hi[:],
                                in1=dinv[0:72, do:do + 200], op=OP.mult)
                        else:
                            nc.gpsimd.memset(adjS_lo[:], 1.0)
                            nc.gpsimd.memset(adjS_hi[:], 1.0)
                        pair.append((adjS_lo, adjS_hi))
                    # ---- 4 GCN layers, pair-interleaved ----
                    for l in range(4):
                        fo = 32 if l < 3 else 1
                        pagg = psG.tile([32, 456], fp32, tag="agg",
                                        name="pagg")
                        py = psY.tile([128, 128], fp32, tag="y", name="py")
                        y = wp.tile([128, 128], fp32, tag="y_s", name="y")
                        for half in range(2):
                            g = 2 * gp + half
                            yo = 64 * half
                            if l == 0:
                                lhs_lo = xT[:, 200 * g:200 * g + 128]
                                lhs_hi = xT[:, 200 * g + 128:200 * g + 200]
                                w_t = w1[:, 0:fo]
                            else:
                                r0 = 32 * (l - 1)
                                c0 = 256 * g
                                lhs_lo = hcat[r0:r0 + 32, c0:c0 + 128]
                                lhs_hi = hcat[r0:r0 + 32, c0 + 128:c0 + 200]
                                w_t = w234[r0:r0 + 32, 0:fo]
                            if "xw" not in skip:
                                nc.tensor.matmul(py[:, yo:yo + fo],
                                                 lhsT=lhs_lo, rhs=w_t,
                                                 start=True, stop=True)
                                nc.tensor.matmul(py[0:72, yo + 32:yo + 32 + fo],
                                                 lhsT=lhs_hi, rhs=w_t,
                                                 start=True, stop=True)
                        # drain both graphs' xw with fused row scale dinv[j]
                        if "xw" not in skip:
                            nc.vector.tensor_tensor(
                                out=y[:], in0=py[:],
                                in1=_apf(dinv[0:128, 200:201],
                                         [[202, 2], [0, 64]]),
                                op=OP.mult)
                        elif "agg" not in skip:
                            nc.gpsimd.memset(y[:], 1.0)
                        if "agg" not in skip:
                            for half in range(2):
                                yo, co = 64 * half, 256 * half
                                adjS_lo, adjS_hi = pair[half]
                                nc.tensor.matmul(pagg[0:fo, co:co + 200],
                                                 lhsT=y[:, yo:yo + fo],
                                                 rhs=adjS_lo[:],
                                                 start=True, stop=False)
                                nc.tensor.matmul(
                                    pagg[0:fo, co:co + 200],
                                    lhsT=y[0:72, yo + 32:yo + 32 + fo],
                                    rhs=adjS_hi[:],
                                    start=False, stop=True)
                            r0 = 32 * l if l < 3 else 96
                            nc.scalar.activation(
                                hcat[r0:r0 + fo, 512 * gp:512 * gp + 456],
                                pagg[0:fo, 0:456], AF.Tanh,
                                bias=bgcn[0:fo, l:l + 1])

                # ---- SortPool: top-30 by h4, descending ----
                h4r = wp.tile([64, 256], fp32, tag="h4r")
                if "sortpool" in skip:
                    nc.gpsimd.memset(topsT[:], 0.5)
                if "sortpool" not in skip:
                    nc.sync.dma_start(h4buf_d.ap(), hcat[96:97, :])
                    nc.sync.dma_start(h4r[:], h4buf_d.ap())
                    nc.vector.memset(h4r[:, 200:256], -1e30)
                    vals = wp.tile([64, 32], fp32, tag="vals")
                    idxu = wp.tile([64, 32], mybir.dt.uint16, tag="idxu")
                    for r in range(4):
                        nc.vector.max(vals[:, 8 * r:8 * r + 8], h4r[:])
                        nc.vector.max_index(idxu[:, 8 * r:8 * r + 8],
                                            vals[:, 8 * r:8 * r + 8], h4r[:])
                        nc.vector.match_replace(h4r[:],
                                                vals[:, 8 * r:8 * r + 8],
                                                h4r[:], -1e30)
                    goff = wp.tile([64, 32], mybir.dt.uint16, tag="goff")
                    nc.gpsimd.iota(goff[:], pattern=[[0, 32]], base=0,
                                   channel_multiplier=256)
                    nc.vector.tensor_tensor(out=idxu[:], in0=idxu[:],
                                            in1=goff[:], op=OP.add)
                    nc.sync.dma_start(
                        idxbuf_d.ap().rearrange("(g k) -> g k", g=GPC),
                        idxu[:].bitcast(mybir.dt.int16))
                    idxw = wp.tile([112, 128], mybir.dt.int16, tag="idxw")
                    nc.sync.dma_start(
                        idxw[0:16, :],
                        idxbuf_d.ap().rearrange("(c p) -> p c", p=16))
                    for rep in range(1, 7):
                        nc.sync.dma_start(idxw[16 * rep:16 * rep + 16, :],
                                          idxw[0:16, :])
                    nc.gpsimd.ap_gather(topsT[:], hcat[:], idxw[:],
                                        channels=112, num_elems=256 * GPC,
                                        d=1, num_idxs=32 * GPC)

            # ---- head: conv1(97->16) -> maxpool2 -> conv2(16->32,k=5)
            #      -> fc 352->128 -> fc 128->1 ----
            with (
                tc.tile_pool(name="head", bufs=2) as hp,
                tc.tile_pool(name="psH", bufs=1, space="PSUM") as psH,
            ):
                c1T = hp.tile([16, 30 * GPC], fp32, tag="c1T")
                for q in range(4):
                    pc1 = psH.tile([16, 480], fp32, tag="c1", bufs=2,
                                   name="pc1")
                    rhs = _apf(topsT[0:97, 512 * q:512 * q + 1],
                               [[32, 16], [1, 30]])
                    nc.tensor.matmul(pc1[:], lhsT=cw1[:], rhs=rhs,
                                     start=True, stop=True)
                    nc.scalar.activation(c1T[:, 480 * q:480 * q + 480],
                                         pc1[:], AF.Relu, bias=cb1[:])
                poolT = hp.tile([16, 15 * GPC], fp32, tag="poolT")
                nc.vector.tensor_tensor(
                    out=_apf(poolT[0:16, 0:1], [[15, GPC], [1, 15]]),
                    in0=_apf(c1T[0:16, 0:1], [[30, GPC], [2, 15]]),
                    in1=_apf(c1T[0:16, 1:2], [[30, GPC], [2, 15]]),
                    op=OP.max)
                c2T = hp.tile([32, 11 * GPC], fp32, tag="c2T")
                for q in range(2):
                    pc2 = psH.tile([32, 352], fp32, tag="c2", bufs=2,
                                   name="pc2")
                    for t in range(5):
                        rhs = _apf(poolT[0:16, 480 * q + t:480 * q + t + 1],
                                   [[15, 32], [1, 11]])
                        nc.tensor.matmul(pc2[:], lhsT=cw2[t][:], rhs=rhs,
                                         start=(t == 0), stop=(t == 4))
                    nc.scalar.activation(c2T[:, 352 * q:352 * q + 352],
                                         pc2[:], AF.Relu, bias=cb2[:])
                # flat[g, o*11+p]: 11 transposes of [32,64] slices
                c2n = hp.tile([64, 352], fp32, tag="c2n")
                for p in range(11):
                    pt = psH.tile([64, 32], fp32, tag="pT", name="pt")
                    nc.tensor.transpose(pt[:],
                                        _apf(c2T[0:32, p:p + 1], [[11, GPC]]),
                                        ident[0:32, 0:32])
                    nc.vector.tensor_copy(
                        _apf(c2n[0:64, p:p + 1], [[11, 32]]), pt[:])
                ft = [hp.tile([128, 64], fp32, tag=f"ft{q}", name=f"ft{q}")
                      for q in range(3)]
                for q in range(3):
                    w = 128 if q < 2 else 96
                    pf = psH.tile([128, 64], fp32, tag="fT", name="pf")
                    nc.tensor.transpose(pf[0:w, :],
                                        c2n[:, 128 * q:128 * q + w],
                                        ident[0:64, 0:64])
                    nc.vector.tensor_copy(ft[q][0:w, :], pf[0:w, :])
                ph = psH.tile([128, 64], fp32, tag="hl")
                for q in range(3):
                    w = 128 if q < 2 else 96
                    nc.tensor.matmul(ph[:], lhsT=lw1[q][0:w, :],
                                     rhs=ft[q][0:w, :],
                                     start=(q == 0), stop=(q == 2))
                hlinT = hp.tile([128, 64], fp32, tag="hlinT")
                nc.scalar.activation(hlinT[:], ph[:], AF.Relu, bias=lb1[:])
                po = psH.tile([1, 64], fp32, tag="po")
                nc.tensor.matmul(po[:], lhsT=lw2[:], rhs=hlinT[:],
                                 start=True, stop=True)
                outT = hp.tile([1, 64], fp32, tag="outT")
                nc.scalar.activation(outT[:], po[:], AF.Sigmoid, bias=lb2[:])
                nc.sync.dma_start(out_d.ap(), outT[:])

    nc.compile()
    return nc


def _prep_inputs(inputs):
    """Shard + densify on host. Returns per-core in_maps."""
    x = np.asarray(inputs["x"], np.float32)
    ei = np.asarray(inputs["edge_index"], np.int64)
    src, dst = ei[0], ei[1]
    g_edge = dst // M
    jl = src - g_edge * M
    il = dst - g_edge * M
    flat = g_edge * (M * M) + jl * M + il
    cnt = np.bincount(flat, minlength=B * M * M).astype(np.float32)
    adj = cnt.reshape(B, M, M)
    adj += np.eye(M, dtype=np.float32)[None]

    w234 = np.concatenate(
        [np.asarray(inputs["W2"], np.float32),
         np.asarray(inputs["W3"], np.float32),
         np.pad(np.asarray(inputs["W4"], np.float32), ((0, 0), (0, 31)))],
        axis=0)  # [96, 32]
    b4p = np.pad(np.asarray(inputs["b4"], np.float32), (0, 31))
    bgcn = np.stack(
        [np.asarray(inputs["b1"], np.float32),
         np.asarray(inputs["b2"], np.float32),
         np.asarray(inputs["b3"], np.float32), b4p], axis=1)  # [32, 4]
    cw1 = np.ascontiguousarray(
        np.asarray(inputs["convW1"], np.float32)[:, 0, :].T)  # [97,16]
    cw2_r = np.asarray(inputs["convW2"], np.float32)  # [32,16,5]
    cw2 = np.ascontiguousarray(
        cw2_r.transpose(2, 1, 0).reshape(80, 32))  # [(t,i),o]
    common = {
        "w1": np.asarray(inputs["W1"], np.float32),
        "w234": np.ascontiguousarray(w234),
        "bgcn": np.ascontiguousarray(bgcn),
        "cw1": cw1,
        "cb1": np.asarray(inputs["convb1"], np.float32).reshape(16, 1),
        "cw2": cw2,
        "cb2": np.asarray(inputs["convb2"], np.float32).reshape(32, 1),
        "lw1": np.asarray(inputs["linW1"], np.float32),
        "lb1": np.asarray(inputs["linb1"], np.float32).reshape(128, 1),
        "lw2": np.asarray(inputs["linW2"], np.float32),
        "lb2": np.asarray(inputs["linb2"], np.float32).reshape(1, 1),
    }
    in_maps = []
    for c in range(NCORES):
        m = dict(common)
        m["x"] = np.ascontiguousarray(x[NPC * c:NPC * (c + 1)])
        m["adj"] = np.ascontiguousarray(
            adj[GPC * c:GPC * (c + 1)].reshape(GPC * M, M))
        in_maps.append(m)
    return in_maps


def _run(inputs, trace=False):
    from concourse import bass_utils
    if "nc" not in _STATE:
        _STATE["nc"] = _build()
    nc = _STATE["nc"]
    in_maps = _prep_inputs(inputs)
    res = bass_utils.run_bass_kernel_spmd(
        nc, in_maps, core_ids=list(range(NCORES)), trace=trace)
    out = np.concatenate([res.results[c]["out"].reshape(GPC)
                          for c in range(NCORES)])
    return out.reshape(B, 1).astype(np.float32), res


def kernel(**inputs) -> np.ndarray:
    out, _ = _run(inputs, trace=False)
    return out



# revision 12
# speedup vs baseline: 2.2286x; 2.2286x over previous
"""DGCNN (4x GCNConv + SortPool + Conv1d head) on 8 Trainium2 NeuronCores.

Data-parallel over graphs: each core owns 64 of the 512 graphs. Per graph
the GCN aggregation is dense: both D^-1/2 norm factors are folded into the
count matrix on-device (cntS2[j,i] = cnt[j,i]*dinv[j]*dinv[i] via a PE
outer product of dinv with itself), so the per-layer chain is just
    xw (bf16 matmul) -> psum copy -> agg (bf16 matmul vs cntS2) -> tanh.
Layer 0 swaps aggregation and W1 (both linear) to avoid transposing x.
All matmuls run in bf16 (4x PE throughput vs fp32); host supplies the
integer graph structure only (bf16 counts, fp32 integer degrees, bf16 x).
SortPool top-30 by h4 via vector max8/max_index; features gathered from a
2-col-slot bf16 hcat with ap_gather d=2; conv/MLP head in bf16.
"""

import numpy as np
import ml_dtypes

B = 512
M = 200
GPC = 64            # graphs per core
NPC = GPC * M
NCORES = 8
K = 30

_STATE = {}


def _apf(base, pairs):
    """AP with the partition dim of `base` and custom free [step,count] pairs."""
    import concourse.bass as bass
    return bass.AP(tensor=base.tensor, offset=base.offset,
                   ap=[list(base.ap[0])] + [list(p) for p in pairs])


def _build():
    import concourse.bass as bass
    import concourse.bacc as bacc
    import concourse.mybir as mybir
    from concourse.tile import TileContext
    from concourse.masks import make_identity

    fp32 = mybir.dt.float32
    bf16 = mybir.dt.bfloat16
    AF = mybir.ActivationFunctionType
    OP = mybir.AluOpType

    nc = bacc.Bacc("TRN2", target_bir_lowering=False, debug=False,
                   num_devices=NCORES)

    cnt_d = nc.dram_tensor("cnt", [GPC * 128, 400], bf16,
                           kind="ExternalInput")
    x_d = nc.dram_tensor("x", [GPC * 128, 256], bf16, kind="ExternalInput")
    degr_d = nc.dram_tensor("degr", [GPC, 200], fp32, kind="ExternalInput")
    w1_d = nc.dram_tensor("w1", [128, 32], bf16, kind="ExternalInput")
    w234_d = nc.dram_tensor("w234", [96, 32], bf16, kind="ExternalInput")
    bgcn_d = nc.dram_tensor("bgcn", [32, 4], fp32, kind="ExternalInput")
    cw1_d = nc.dram_tensor("cw1", [97, 16], bf16, kind="ExternalInput")
    cb1_d = nc.dram_tensor("cb1", [16, 1], fp32, kind="ExternalInput")
    cw2_d = nc.dram_tensor("cw2", [80, 32], bf16, kind="ExternalInput")
    cb2_d = nc.dram_tensor("cb2", [32, 1], fp32, kind="ExternalInput")
    lw1_d = nc.dram_tensor("lw1", [352, 128], bf16, kind="ExternalInput")
    lb1_d = nc.dram_tensor("lb1", [128, 1], fp32, kind="ExternalInput")
    lw2_d = nc.dram_tensor("lw2", [128, 1], bf16, kind="ExternalInput")
    lb2_d = nc.dram_tensor("lb2", [1, 1], fp32, kind="ExternalInput")

    h4buf_d = nc.dram_tensor("h4buf", [GPC * 200], bf16, kind="Internal")
    dinvbuf_d = nc.dram_tensor("dinvbuf", [GPC * 200], bf16, kind="Internal")
    valbuf_d = nc.dram_tensor("valbuf", [GPC * 32], bf16, kind="Internal")
    idxbuf_d = nc.dram_tensor("idxbuf", [GPC * 32], mybir.dt.int16,
                              kind="Internal")
    out_d = nc.dram_tensor("out", [1, GPC], fp32, kind="ExternalOutput")

    with TileContext(nc) as tc:
        with tc.tile_pool(name="const", bufs=1) as cp:
            ident16 = cp.tile([64, 64], bf16)
            make_identity(nc, ident16[:])
            w1 = cp.tile([128, 32], bf16)
            nc.sync.dma_start(w1[:], w1_d.ap())
            w234 = cp.tile([96, 32], bf16)
            nc.sync.dma_start(w234[:], w234_d.ap())
            bgcn = cp.tile([32, 4], fp32)
            nc.sync.dma_start(bgcn[:], bgcn_d.ap())
            cw1 = cp.tile([96, 16], bf16)
            nc.sync.dma_start(cw1[:], cw1_d.ap()[0:96, :])
            cw1b = cp.tile([1, 16], bf16)
            nc.sync.dma_start(cw1b[:], cw1_d.ap()[96:97, :])
            cb1 = cp.tile([16, 1], fp32)
            nc.sync.dma_start(cb1[:], cb1_d.ap())
            cw2 = [cp.tile([16, 32], bf16, tag=f"cw2_{t}", name=f"cw2_{t}")
                   for t in range(5)]
            for t in range(5):
                nc.sync.dma_start(cw2[t][:], cw2_d.ap()[16 * t:16 * t + 16, :])
            cb2 = cp.tile([32, 1], fp32)
            nc.sync.dma_start(cb2[:], cb2_d.ap())
            lw1 = [cp.tile([128, 128], bf16, tag=f"lw1_{q}", name=f"lw1_{q}")
                   for q in range(3)]
            nc.sync.dma_start(lw1[0][:], lw1_d.ap()[0:128, :])
            nc.sync.dma_start(lw1[1][:], lw1_d.ap()[128:256, :])
            nc.sync.dma_start(lw1[2][0:96, :], lw1_d.ap()[256:352, :])
            lb1 = cp.tile([128, 1], fp32)
            nc.sync.dma_start(lb1[:], lb1_d.ap())
            lw2 = cp.tile([128, 1], bf16)
            nc.sync.dma_start(lw2[:], lw2_d.ap())
            lb2 = cp.tile([1, 1], fp32)
            nc.sync.dma_start(lb2[:], lb2_d.ap())

            # dinv = rsqrt(deg), once for all 64 graphs
            degr = cp.tile([GPC, 200], fp32)
            nc.scalar.dma_start(degr[:], degr_d.ap())
            rec = cp.tile([GPC, 200], fp32)
            nc.vector.reciprocal(rec[:], degr[:])
            dinvr = cp.tile([GPC, 200], fp32)
            nc.scalar.activation(dinvr[:], rec[:], AF.Sqrt)
            dinv16 = cp.tile([GPC, 200], bf16)
            nc.vector.tensor_copy(dinv16[:], dinvr[:])
            # relayout to a single row (matmul lhsT needs base partition 0)
            nc.sync.dma_start(
                dinvbuf_d.ap().rearrange("(g i) -> g i", g=GPC), dinv16[:])
            dinvrow = cp.tile([1, GPC * 200], bf16)
            nc.sync.dma_start(dinvrow[:],
                              dinvbuf_d.ap().rearrange("(p e) -> p e", p=1))

            # hcat16: rows 0:96 = h1|h2|h3, 2-col slots: col = 512*g + 2*i
            hcat16 = cp.tile([96, 512 * GPC], bf16)
            # h4 compact (for SortPool ranking) + per-rank values
            h4c = cp.tile([1, 200 * GPC], bf16)
            topsT = cp.tile([96, 64 * GPC], bf16)
            valrow = cp.tile([1, 32 * GPC], bf16)

            with (
                tc.tile_pool(name="work", bufs=2) as wp,
                tc.tile_pool(name="psD", bufs=2, space="PSUM") as psD,
                tc.tile_pool(name="psA", bufs=1, space="PSUM") as psA,
                tc.tile_pool(name="psH", bufs=3, space="PSUM") as psH,
                tc.tile_pool(name="psY", bufs=2, space="PSUM") as psY,
            ):
                for gp in range(GPC // 2):
                    g0 = 2 * gp
                    cnt16 = wp.tile([128, 800], bf16, tag="cnt16", bufs=3,
                                    name="cnt16")
                    nc.sync.dma_start(
                        cnt16[:],
                        _apf(cnt_d.ap()[128 * g0:128 * g0 + 128, :],
                             [[128 * 400, 2], [1, 400]]))
                    x16 = wp.tile([128, 512], bf16, tag="x16", bufs=3,
                                  name="x16")
                    nc.scalar.dma_start(
                        x16[:],
                        _apf(x_d.ap()[128 * g0:128 * g0 + 128, :],
                             [[128 * 256, 2], [1, 256]]))

                    # cntS2 = cnt * dinv[j] * dinv[i] (both graphs)
                    cntS2 = wp.tile([128, 800], bf16, tag="cntS2", bufs=2,
                                    name="cntS2")
                    for h in range(2):
                        g = g0 + h
                        d2 = psD.tile([128, 400], fp32, tag="d2", name="d2")
                        nc.tensor.matmul(
                            d2[0:128, 0:200],
                            lhsT=dinvrow[0:1, 200 * g:200 * g + 128],
                            rhs=dinvrow[0:1, 200 * g:200 * g + 200],
                            start=True, stop=True)
                        nc.tensor.matmul(
                            d2[0:72, 200:400],
                            lhsT=dinvrow[0:1, 200 * g + 128:200 * g + 200],
                            rhs=dinvrow[0:1, 200 * g:200 * g + 200],
                            start=True, stop=True)
                        nc.vector.tensor_tensor(
                            out=cntS2[:, 400 * h:400 * h + 400],
                            in0=cnt16[:, 400 * h:400 * h + 400],
                            in1=d2[:], op=OP.mult)

                    # layer 0: aggregate x first, then W1
                    pax = psA.tile([128, 456], fp32, tag="pax", name="pax")
                    for h in range(2):
                        co = 256 * h
                        nc.tensor.matmul(pax[0:128, co:co + 200],
                                         lhsT=x16[0:128,
                                                  256 * h:256 * h + 128],
                                         rhs=cntS2[0:128,
                                                   400 * h:400 * h + 200],
                                         start=True, stop=False)
                        nc.tensor.matmul(pax[0:128, co:co + 200],
                                         lhsT=x16[0:72,
                                                  256 * h + 128:256 * h + 256],
                                         rhs=cntS2[0:72,
                                                   400 * h + 200:400 * h + 400],
                                         start=False, stop=True)
                    ax16 = wp.tile([128, 456], bf16, tag="ax16", bufs=2,
                                   name="ax16")
                    nc.scalar.copy(ax16[:], pax[:])
                    ph1 = psH.tile([32, 456], fp32, tag="ph", name="ph1")
                    for h in range(2):
                        co = 256 * h
                        nc.tensor.matmul(ph1[0:32, co:co + 200],
                                         lhsT=w1[:],
                                         rhs=ax16[0:128, co:co + 200],
                                         start=True, stop=True)
                    nc.scalar.activation(
                        _apf(hcat16[0:32, 1024 * gp:1024 * gp + 1],
                             [[512, 2], [2, 200]]),
                        _apf(ph1[0:32, 0:1], [[256, 2], [1, 200]]),
                        AF.Tanh, bias=bgcn[0:32, 0:1])

                    # layers 1..3
                    for l in range(1, 4):
                        fo = 32 if l < 3 else 1
                        r0 = 32 * (l - 1)
                        ppy = psY.tile([128, 128], fp32, tag="ppy",
                                       name="ppy")
                        for h in range(2):
                            g = g0 + h
                            yo = 64 * h
                            nc.tensor.matmul(
                                ppy[0:128, yo:yo + fo],
                                lhsT=_apf(hcat16[r0:r0 + 32, 512 * g:512 * g + 1],
                                          [[2, 128]]),
                                rhs=w234[r0:r0 + 32, 0:fo],
                                start=True, stop=True)
                            nc.tensor.matmul(
                                ppy[0:72, yo + 32:yo + 32 + fo],
                                lhsT=_apf(hcat16[r0:r0 + 32,
                                                 512 * g + 256:512 * g + 257],
                                          [[2, 72]]),
                                rhs=w234[r0:r0 + 32, 0:fo],
                                start=True, stop=True)
                        y16 = wp.tile([128, 128], bf16, tag="y16", bufs=3,
                                      name="y16")
                        nc.vector.tensor_copy(y16[:], ppy[:])
                        pagg = psH.tile([32, 456], fp32, tag="ph",
                                        name="pagg")
                        for h in range(2):
                            co = 256 * h
                            nc.tensor.matmul(
                                pagg[0:fo, co:co + 200],
                                lhsT=y16[0:128, 64 * h:64 * h + fo],
                                rhs=cntS2[0:128, 400 * h:400 * h + 200],
                                start=True, stop=False)
                            nc.tensor.matmul(
                                pagg[0:fo, co:co + 200],
                                lhsT=y16[0:72, 64 * h + 32:64 * h + 32 + fo],
                                rhs=cntS2[0:72, 400 * h + 200:400 * h + 400],
                                start=False, stop=True)
                        if l < 3:
                            nc.scalar.activation(
                                _apf(hcat16[32 * l:32 * l + 32,
                                            1024 * gp:1024 * gp + 1],
                                     [[512, 2], [2, 200]]),
                                _apf(pagg[0:32, 0:1], [[256, 2], [1, 200]]),
                                AF.Tanh, bias=bgcn[0:32, l:l + 1])
                        else:
                            nc.scalar.activation(
                                _apf(h4c[0:1, 400 * gp:400 * gp + 1],
                                     [[200, 2], [1, 200]]),
                                _apf(pagg[0:1, 0:1], [[256, 2], [1, 200]]),
                                AF.Tanh, bias=bgcn[0:1, 3:4])

                # ---- SortPool: top-30 by h4, descending ----
                nc.sync.dma_start(h4buf_d.ap(), h4c[:])
                h4r = wp.tile([64, 256], bf16, tag="h4r")
                nc.vector.memset(h4r[:, 200:256], -1e30)
                nc.sync.dma_start(
                    h4r[:, 0:200],
                    h4buf_d.ap().rearrange("(g i) -> g i", g=GPC))
                vals = wp.tile([64, 32], bf16, tag="vals")
                idxu = wp.tile([64, 32], mybir.dt.uint16, tag="idxu")
                for r in range(4):
                    nc.vector.max(vals[:, 8 * r:8 * r + 8], h4r[:])
                    nc.vector.max_index(idxu[:, 8 * r:8 * r + 8],
                                        vals[:, 8 * r:8 * r + 8], h4r[:])
                    nc.vector.match_replace(h4r[:],
                                            vals[:, 8 * r:8 * r + 8],
                                            h4r[:], -1e30)
                nc.sync.dma_start(
                    valbuf_d.ap().rearrange("(g k) -> g k", g=GPC), vals[:])
                nc.sync.dma_start(valrow[:],
                                  valbuf_d.ap().rearrange("(p e) -> p e", p=1))
                goff = wp.tile([64, 32], mybir.dt.uint16, tag="goff")
                nc.gpsimd.iota(goff[:], pattern=[[0, 32]], base=0,
                               channel_multiplier=256)
                nc.vector.tensor_tensor(out=idxu[:], in0=idxu[:],
                                        in1=goff[:], op=OP.add)
                nc.sync.dma_start(
                    idxbuf_d.ap().rearrange("(g k) -> g k", g=GPC),
                    idxu[:].bitcast(mybir.dt.int16))
                idxw = wp.tile([96, 128], mybir.dt.int16, tag="idxw")
                nc.sync.dma_start(
                    idxw[0:16, :],
                    idxbuf_d.ap().rearrange("(c p) -> p c", p=16))
                for rep in range(1, 6):
                    nc.sync.dma_start(idxw[16 * rep:16 * rep + 16, :],
                                      idxw[0:16, :])
                nc.gpsimd.ap_gather(topsT[:], hcat16[:], idxw[:],
                                    channels=96, num_elems=256 * GPC,
                                    d=2, num_idxs=32 * GPC)

            # ---- head: conv1(97->16) -> maxpool2 -> conv2(16->32,k=5)
            #      -> fc 352->128 -> fc 128->1 ----
            with (
                tc.tile_pool(name="head", bufs=2) as hp,
                tc.tile_pool(name="psHd", bufs=1, space="PSUM") as psHd,
            ):
                c1T = hp.tile([16, 30 * GPC], bf16, tag="c1T")
                for q in range(4):
                    pc1 = psHd.tile([16, 480], fp32, tag="c1", bufs=2,
                                    name="pc1")
                    rhs = _apf(topsT[0:96, 1024 * q:1024 * q + 1],
                               [[64, 16], [2, 30]])
                    nc.tensor.matmul(pc1[:], lhsT=cw1[:], rhs=rhs,
                                     start=True, stop=False)
                    rhsv = _apf(valrow[0:1, 512 * q:512 * q + 1],
                                [[32, 16], [1, 30]])
                    nc.tensor.matmul(pc1[:], lhsT=cw1b[:], rhs=rhsv,
                                     start=False, stop=True)
                    nc.scalar.activation(c1T[:, 480 * q:480 * q + 480],
                                         pc1[:], AF.Relu, bias=cb1[:])
                poolT = hp.tile([16, 15 * GPC], bf16, tag="poolT")
                nc.vector.tensor_tensor(
                    out=_apf(poolT[0:16, 0:1], [[15, GPC], [1, 15]]),
                    in0=_apf(c1T[0:16, 0:1], [[30, GPC], [2, 15]]),
                    in1=_apf(c1T[0:16, 1:2], [[30, GPC], [2, 15]]),
                    op=OP.max)
                c2T = hp.tile([32, 11 * GPC], bf16, tag="c2T")
                for q in range(2):
                    pc2 = psHd.tile([32, 352], fp32, tag="c2", bufs=2,
                                    name="pc2")
                    for t in range(5):
                        rhs = _apf(poolT[0:16, 480 * q + t:480 * q + t + 1],
                                   [[15, 32], [1, 11]])
                        nc.tensor.matmul(pc2[:], lhsT=cw2[t][:], rhs=rhs,
                                         start=(t == 0), stop=(t == 4))
                    nc.scalar.activation(c2T[:, 352 * q:352 * q + 352],
                                         pc2[:], AF.Relu, bias=cb2[:])
                # flat[g, o*11+p]: 11 transposes of [32,64] slices
                c2n = hp.tile([64, 352], bf16, tag="c2n")
                for p in range(11):
                    pt = psHd.tile([64, 32], bf16, tag="pT", name="pt")
                    nc.tensor.transpose(pt[:],
                                        _apf(c2T[0:32, p:p + 1], [[11, GPC]]),
                                        ident16[0:32, 0:32])
                    nc.vector.tensor_copy(
                        _apf(c2n[0:64, p:p + 1], [[11, 32]]), pt[:])
                ft = [hp.tile([128, 64], bf16, tag=f"ft{q}", name=f"ft{q}")
                      for q in range(3)]
                for q in range(3):
                    w = 128 if q < 2 else 96
                    pf = psHd.tile([128, 64], bf16, tag="fT", name="pf")
                    nc.tensor.transpose(pf[0:w, :],
                                        c2n[:, 128 * q:128 * q + w],
                                        ident16[:])
                    nc.vector.tensor_copy(ft[q][0:w, :], pf[0:w, :])
                ph = psHd.tile([128, 64], fp32, tag="hl")
                for q in range(3):
                    w = 128 if q < 2 else 96
                    nc.tensor.matmul(ph[:], lhsT=lw1[q][0:w, :],
                                     rhs=ft[q][0:w, :],
                                     start=(q == 0), stop=(q == 2))
                hlinT = hp.tile([128, 64], bf16, tag="hlinT")
                nc.scalar.activation(hlinT[:], ph[:], AF.Relu, bias=lb1[:])
                po = psHd.tile([1, 64], fp32, tag="po")
                nc.tensor.matmul(po[:], lhsT=lw2[:], rhs=hlinT[:],
                                 start=True, stop=True)
                outT = hp.tile([1, 64], fp32, tag="outT")
                nc.scalar.activation(outT[:], po[:], AF.Sigmoid, bias=lb2[:])
                nc.sync.dma_start(out_d.ap(), outT[:])

    nc.compile()
    return nc


def _prep_inputs(inputs):
    """Shard + densify on host (integer graph structure only)."""
    bf = ml_dtypes.bfloat16
    x = np.asarray(inputs["x"], np.float32)
    ei = np.asarray(inputs["edge_index"], np.int64)
    src, dst = ei[0], ei[1]
    g_edge = dst // M
    jl = src - g_edge * M
    il = dst - g_edge * M
    flat = g_edge * (M * M) + jl * M + il
    cnt = np.bincount(flat, minlength=B * M * M).astype(np.float32)
    adj = cnt.reshape(B, M, M)
    adj += np.eye(M, dtype=np.float32)[None]

    # pack counts: [B, 128, 400] (j-lo cols 0:200, j-hi cols 200:400)
    cntp = np.zeros((B, 128, 400), np.float32)
    cntp[:, :, 0:200] = adj[:, 0:128, :]
    cntp[:, 0:72, 200:400] = adj[:, 128:200, :]
    cntp = cntp.astype(bf)

    # pack x: [B, 128, 256] (node-lo cols 0:128, node-hi cols 128:256)
    xr = x.reshape(B, M, 128)
    xp = np.zeros((B, 128, 256), np.float32)
    xp[:, :, 0:128] = xr[:, 0:128, :]
    xp[:, 0:72, 128:256] = xr[:, 128:200, :]
    xp = xp.astype(bf)

    degr = adj.sum(axis=1)  # [B, 200] integer-valued fp32 in-degree+1

    w234 = np.concatenate(
        [np.asarray(inputs["W2"], np.float32),
         np.asarray(inputs["W3"], np.float32),
         np.pad(np.asarray(inputs["W4"], np.float32), ((0, 0), (0, 31)))],
        axis=0)
    b4p = np.pad(np.asarray(inputs["b4"], np.float32), (0, 31))
    bgcn = np.stack(
        [np.asarray(inputs["b1"], np.float32),
         np.asarray(inputs["b2"], np.float32),
         np.asarray(inputs["b3"], np.float32), b4p], axis=1)
    cw1 = np.ascontiguousarray(
        np.asarray(inputs["convW1"], np.float32)[:, 0, :].T)
    cw2_r = np.asarray(inputs["convW2"], np.float32)
    cw2 = np.ascontiguousarray(cw2_r.transpose(2, 1, 0).reshape(80, 32))
    common = {
        "w1": np.asarray(inputs["W1"], np.float32).astype(bf),
        "w234": np.ascontiguousarray(w234).astype(bf),
        "bgcn": np.ascontiguousarray(bgcn),
        "cw1": cw1.astype(bf),
        "cb1": np.asarray(inputs["convb1"], np.float32).reshape(16, 1),
        "cw2": cw2.astype(bf),
        "cb2": np.asarray(inputs["convb2"], np.float32).reshape(32, 1),
        "lw1": np.asarray(inputs["linW1"], np.float32).astype(bf),
        "lb1": np.asarray(inputs["linb1"], np.float32).reshape(128, 1),
        "lw2": np.asarray(inputs["linW2"], np.float32).astype(bf),
        "lb2": np.asarray(inputs["linb2"], np.float32).reshape(1, 1),
    }
    in_maps = []
    for c in range(NCORES):
        m = dict(common)
        m["cnt"] = np.ascontiguousarray(
            cntp[GPC * c:GPC * (c + 1)].reshape(GPC * 128, 400))
        m["x"] = np.ascontiguousarray(
            xp[GPC * c:GPC * (c + 1)].reshape(GPC * 128, 256))
        m["degr"] = np.ascontiguousarray(degr[GPC * c:GPC * (c + 1)])
        in_maps.append(m)
    return in_maps


def _run(inputs, trace=False):
    from concourse import bass_utils
    if "nc" not in _STATE:
        _STATE["nc"] = _build()
    nc = _STATE["nc"]
    in_maps = _prep_inputs(inputs)
    res = bass_utils.run_bass_kernel_spmd(
        nc, in_maps, core_ids=list(range(NCORES)), trace=trace)
    out = np.concatenate([res.results[c]["out"].reshape(GPC)
                          for c in range(NCORES)])
    return out.reshape(B, 1).astype(np.float32), res


def kernel(**inputs) -> np.ndarray:
    out, _ = _run(inputs, trace=False)
    return out


# revision 15
# speedup vs baseline: 2.9496x; 1.3235x over previous
"""DGCNN (4x GCNConv + SortPool + Conv1d head) on 8 Trainium2 NeuronCores.

Data-parallel over graphs: each core owns 64 of the 512 graphs. Per graph
the GCN aggregation is dense: both D^-1/2 norm factors are folded into the
count matrix on-device (cntS2[j,i] = cnt[j,i]*dinv[j]*dinv[i] via a PE
outer product of dinv with itself), so the per-layer chain is just
    xw (bf16 matmul) -> psum copy -> agg (bf16 matmul vs cntS2) -> tanh.
Layer 0 swaps aggregation and W1 (both linear) to avoid transposing x.
All matmuls run in bf16 (4x PE throughput vs fp32); host supplies the
integer graph structure only (bf16 counts, fp32 integer degrees, bf16 x).
SortPool top-30 by h4 via vector max8/max_index; features gathered from a
2-col-slot bf16 hcat with ap_gather d=2; conv/MLP head in bf16.
"""

import numpy as np
import ml_dtypes

B = 512
M = 200
GPC = 64            # graphs per core
NPC = GPC * M
NCORES = 8
K = 30

_STATE = {}


def _apf(base, pairs):
    """AP with the partition dim of `base` and custom free [step,count] pairs."""
    import concourse.bass as bass
    return bass.AP(tensor=base.tensor, offset=base.offset,
                   ap=[list(base.ap[0])] + [list(p) for p in pairs])


def _build():
    import concourse.bass as bass
    import concourse.bacc as bacc
    import concourse.mybir as mybir
    from concourse.tile import TileContext
    from concourse.masks import make_identity

    fp32 = mybir.dt.float32
    bf16 = mybir.dt.bfloat16
    AF = mybir.ActivationFunctionType
    OP = mybir.AluOpType

    nc = bacc.Bacc("TRN2", target_bir_lowering=False, debug=False,
                   num_devices=NCORES)

    cnt_d = nc.dram_tensor("cnt", [GPC * 128, 400], bf16,
                           kind="ExternalInput")
    x_d = nc.dram_tensor("x", [GPC * 128, 256], bf16, kind="ExternalInput")
    degr_d = nc.dram_tensor("degr", [GPC, 200], fp32, kind="ExternalInput")
    w1_d = nc.dram_tensor("w1", [128, 32], bf16, kind="ExternalInput")
    w234_d = nc.dram_tensor("w234", [96, 32], bf16, kind="ExternalInput")
    bgcn_d = nc.dram_tensor("bgcn", [32, 4], fp32, kind="ExternalInput")
    cw1_d = nc.dram_tensor("cw1", [97, 16], bf16, kind="ExternalInput")
    cb1_d = nc.dram_tensor("cb1", [16, 1], fp32, kind="ExternalInput")
    cw2_d = nc.dram_tensor("cw2", [80, 32], bf16, kind="ExternalInput")
    cb2_d = nc.dram_tensor("cb2", [32, 1], fp32, kind="ExternalInput")
    lw1_d = nc.dram_tensor("lw1", [352, 128], bf16, kind="ExternalInput")
    lb1_d = nc.dram_tensor("lb1", [128, 1], fp32, kind="ExternalInput")
    lw2_d = nc.dram_tensor("lw2", [128, 1], bf16, kind="ExternalInput")
    lb2_d = nc.dram_tensor("lb2", [1, 1], fp32, kind="ExternalInput")

    h4buf_d = nc.dram_tensor("h4buf", [GPC * 200], bf16, kind="Internal")
    dinvbuf_d = nc.dram_tensor("dinvbuf", [GPC * 200], bf16, kind="Internal")
    valbuf_d = nc.dram_tensor("valbuf", [GPC * 32], bf16, kind="Internal")
    idxbuf_d = nc.dram_tensor("idxbuf", [GPC * 32], mybir.dt.int16,
                              kind="Internal")
    out_d = nc.dram_tensor("out", [1, GPC], fp32, kind="ExternalOutput")

    with TileContext(nc) as tc:
        with tc.tile_pool(name="const", bufs=1) as cp:
            ident16 = cp.tile([64, 64], bf16)
            make_identity(nc, ident16[:])
            # Warm the GpSimd Q7 gather/iota ucode early: the program load
            # costs ~57us and otherwise lands right before the real gather.
            dgi = cp.tile([16, 1], mybir.dt.int16)
            nc.vector.memset(dgi[:], 0)
            dgo = cp.tile([16, 32], bf16)
            nc.gpsimd.ap_gather(dgo[:], ident16[0:16, 0:64], dgi[:],
                                channels=16, num_elems=32, d=2, num_idxs=16)
            goff = cp.tile([64, 32], mybir.dt.uint16)
            nc.gpsimd.iota(goff[:], pattern=[[0, 32]], base=0,
                           channel_multiplier=256)
            w1 = cp.tile([128, 32], bf16)
            nc.sync.dma_start(w1[:], w1_d.ap())
            w234 = cp.tile([96, 32], bf16)
            nc.sync.dma_start(w234[:], w234_d.ap())
            bgcn = cp.tile([32, 4], fp32)
            nc.sync.dma_start(bgcn[:], bgcn_d.ap())
            cw1 = cp.tile([96, 16], bf16)
            nc.sync.dma_start(cw1[:], cw1_d.ap()[0:96, :])
            cw1b = cp.tile([1, 16], bf16)
            nc.sync.dma_start(cw1b[:], cw1_d.ap()[96:97, :])
            cb1 = cp.tile([16, 1], fp32)
            nc.sync.dma_start(cb1[:], cb1_d.ap())
            cw2 = [cp.tile([16, 32], bf16, tag=f"cw2_{t}", name=f"cw2_{t}")
                   for t in range(5)]
            for t in range(5):
                nc.sync.dma_start(cw2[t][:], cw2_d.ap()[16 * t:16 * t + 16, :])
            cb2 = cp.tile([32, 1], fp32)
            nc.sync.dma_start(cb2[:], cb2_d.ap())
            lw1 = [cp.tile([128, 128], bf16, tag=f"lw1_{q}", name=f"lw1_{q}")
                   for q in range(3)]
            nc.sync.dma_start(lw1[0][:], lw1_d.ap()[0:128, :])
            nc.sync.dma_start(lw1[1][:], lw1_d.ap()[128:256, :])
            nc.sync.dma_start(lw1[2][0:96, :], lw1_d.ap()[256:352, :])
            lb1 = cp.tile([128, 1], fp32)
            nc.sync.dma_start(lb1[:], lb1_d.ap())
            lw2 = cp.tile([128, 1], bf16)
            nc.sync.dma_start(lw2[:], lw2_d.ap())
            lb2 = cp.tile([1, 1], fp32)
            nc.sync.dma_start(lb2[:], lb2_d.ap())

            # dinv = rsqrt(deg), once for all 64 graphs
            degr = cp.tile([GPC, 200], fp32)
            nc.scalar.dma_start(degr[:], degr_d.ap())
            rec = cp.tile([GPC, 200], fp32)
            nc.vector.reciprocal(rec[:], degr[:])
            dinvr = cp.tile([GPC, 200], fp32)
            nc.scalar.activation(dinvr[:], rec[:], AF.Sqrt)
            dinv16 = cp.tile([GPC, 200], bf16)
            nc.vector.tensor_copy(dinv16[:], dinvr[:])
            # relayout to a single row (matmul lhsT needs base partition 0)
            nc.sync.dma_start(
                dinvbuf_d.ap().rearrange("(g i) -> g i", g=GPC), dinv16[:])
            dinvrow = cp.tile([1, GPC * 200], bf16)
            nc.sync.dma_start(dinvrow[:],
                              dinvbuf_d.ap().rearrange("(p e) -> p e", p=1))

            # hcat16: rows 0:96 = h1|h2|h3, 2-col slots: col = 512*g + 2*i
            hcat16 = cp.tile([96, 512 * GPC], bf16)
            # h4 compact (for SortPool ranking) + per-rank values
            h4c = cp.tile([1, 200 * GPC], bf16)
            topsT = cp.tile([96, 64 * GPC], bf16)
            valrow = cp.tile([1, 32 * GPC], bf16)

            with (
                tc.tile_pool(name="work", bufs=2) as wp,
                tc.tile_pool(name="psD", bufs=2, space="PSUM") as psD,
                tc.tile_pool(name="psA", bufs=2, space="PSUM") as psA,
                tc.tile_pool(name="psH", bufs=2, space="PSUM") as psH,
                tc.tile_pool(name="psY", bufs=2, space="PSUM") as psY,
            ):
                # software-pipelined in groups of GRP pairs: each phase is
                # emitted for the whole group so every engine FIFO always
                # holds independent work from other pairs.
                GRP = 8
                for grp in range(GPC // 2 // GRP):
                    pairs = list(range(GRP * grp, GRP * (grp + 1)))
                    cnt16s, x16s, cntS2s = {}, {}, {}
                    for gp in pairs:
                        g0 = 2 * gp
                        cnt16s[gp] = wp.tile([128, 800], bf16, tag="cnt16",
                                             bufs=2 * GRP, name="cnt16")
                        nc.sync.dma_start(
                            cnt16s[gp][:],
                            _apf(cnt_d.ap()[128 * g0:128 * g0 + 128, :],
                                 [[128 * 400, 2], [1, 400]]))
                        x16s[gp] = wp.tile([128, 512], bf16, tag="x16",
                                           bufs=2 * GRP, name="x16")
                        nc.scalar.dma_start(
                            x16s[gp][:],
                            _apf(x_d.ap()[128 * g0:128 * g0 + 128, :],
                                 [[128 * 256, 2], [1, 256]]))

                    # cntS2 = cnt * dinv[j] * dinv[i] (both graphs)
                    for gp in pairs:
                        g0 = 2 * gp
                        cntS2s[gp] = wp.tile([128, 800], bf16, tag="cntS2",
                                             bufs=2 * GRP, name="cntS2")
                        for h in range(2):
                            g = g0 + h
                            d2 = psD.tile([128, 400], fp32, tag="d2",
                                          name="d2")
                            nc.tensor.matmul(
                                d2[0:128, 0:200],
                                lhsT=dinvrow[0:1, 200 * g:200 * g + 128],
                                rhs=dinvrow[0:1, 200 * g:200 * g + 200],
                                start=True, stop=True)
                            nc.tensor.matmul(
                                d2[0:72, 200:400],
                                lhsT=dinvrow[0:1, 200 * g + 128:200 * g + 200],
                                rhs=dinvrow[0:1, 200 * g:200 * g + 200],
                                start=True, stop=True)
                            nc.vector.tensor_tensor(
                                out=cntS2s[gp][:, 400 * h:400 * h + 400],
                                in0=cnt16s[gp][:, 400 * h:400 * h + 400],
                                in1=d2[:], op=OP.mult)

                    # layer 0: aggregate x first, then W1
                    for gp in pairs:
                        cntS2 = cntS2s[gp]
                        x16 = x16s[gp]
                        pax = psA.tile([128, 456], fp32, tag="pax",
                                       name="pax")
                        for h in range(2):
                            co = 256 * h
                            nc.tensor.matmul(
                                pax[0:128, co:co + 200],
                                lhsT=x16[0:128, 256 * h:256 * h + 128],
                                rhs=cntS2[0:128, 400 * h:400 * h + 200],
                                start=True, stop=False)
                            nc.tensor.matmul(
                                pax[0:128, co:co + 200],
                                lhsT=x16[0:72, 256 * h + 128:256 * h + 256],
                                rhs=cntS2[0:72, 400 * h + 200:400 * h + 400],
                                start=False, stop=True)
                        ax16 = wp.tile([128, 456], bf16, tag="ax16", bufs=3,
                                       name="ax16")
                        nc.vector.tensor_copy(ax16[:], pax[:])
                        ph1 = psH.tile([32, 456], fp32, tag="ph", name="ph1")
                        for h in range(2):
                            co = 256 * h
                            nc.tensor.matmul(ph1[0:32, co:co + 200],
                                             lhsT=w1[:],
                                             rhs=ax16[0:128, co:co + 200],
                                             start=True, stop=True)
                        nc.scalar.activation(
                            _apf(hcat16[0:32, 1024 * gp:1024 * gp + 1],
                                 [[512, 2], [2, 200]]),
                            _apf(ph1[0:32, 0:1], [[256, 2], [1, 200]]),
                            AF.Tanh, bias=bgcn[0:32, 0:1])

                    # layers 1..3
                    for l in range(1, 4):
                        fo = 32 if l < 3 else 1
                        r0 = 32 * (l - 1)
                        for gp in pairs:
                            g0 = 2 * gp
                            cntS2 = cntS2s[gp]
                            ppy = psY.tile([128, 128], fp32, tag="ppy",
                                           name="ppy")
                            for h in range(2):
                                g = g0 + h
                                yo = 64 * h
                                nc.tensor.matmul(
                                    ppy[0:128, yo:yo + fo],
                                    lhsT=_apf(hcat16[r0:r0 + 32,
                                                     512 * g:512 * g + 1],
                                              [[2, 128]]),
                                    rhs=w234[r0:r0 + 32, 0:fo],
                                    start=True, stop=True)
                                nc.tensor.matmul(
                                    ppy[0:72, yo + 32:yo + 32 + fo],
                                    lhsT=_apf(hcat16[r0:r0 + 32,
                                                     512 * g + 256:512 * g + 257],
                                              [[2, 72]]),
                                    rhs=w234[r0:r0 + 32, 0:fo],
                                    start=True, stop=True)
                            y16 = wp.tile([128, 128], bf16, tag="y16",
                                          bufs=4, name="y16")
                            nc.vector.tensor_copy(y16[:], ppy[:])
                            pagg = psH.tile([32, 456], fp32, tag="ph",
                                            name="pagg")
                            for h in range(2):
                                co = 256 * h
                                nc.tensor.matmul(
                                    pagg[0:fo, co:co + 200],
                                    lhsT=y16[0:128, 64 * h:64 * h + fo],
                                    rhs=cntS2[0:128, 400 * h:400 * h + 200],
                                    start=True, stop=False)
                                nc.tensor.matmul(
                                    pagg[0:fo, co:co + 200],
                                    lhsT=y16[0:72,
                                             64 * h + 32:64 * h + 32 + fo],
                                    rhs=cntS2[0:72,
                                              400 * h + 200:400 * h + 400],
                                    start=False, stop=True)
                            if l < 3:
                                nc.scalar.activation(
                                    _apf(hcat16[32 * l:32 * l + 32,
                                                1024 * gp:1024 * gp + 1],
                                         [[512, 2], [2, 200]]),
                                    _apf(pagg[0:32, 0:1],
                                         [[256, 2], [1, 200]]),
                                    AF.Tanh, bias=bgcn[0:32, l:l + 1])
                            else:
                                nc.scalar.activation(
                                    _apf(h4c[0:1, 400 * gp:400 * gp + 1],
                                         [[200, 2], [1, 200]]),
                                    _apf(pagg[0:1, 0:1],
                                         [[256, 2], [1, 200]]),
                                    AF.Tanh, bias=bgcn[0:1, 3:4])

                # ---- SortPool: top-30 by h4, descending ----
                nc.sync.dma_start(h4buf_d.ap(), h4c[:])
                h4r = wp.tile([64, 256], bf16, tag="h4r")
                nc.vector.memset(h4r[:, 200:256], -1e30)
                nc.sync.dma_start(
                    h4r[:, 0:200],
                    h4buf_d.ap().rearrange("(g i) -> g i", g=GPC))
                vals = wp.tile([64, 32], bf16, tag="vals")
                idxu = wp.tile([64, 32], mybir.dt.uint16, tag="idxu")
                for r in range(4):
                    nc.vector.max(vals[:, 8 * r:8 * r + 8], h4r[:])
                    nc.vector.max_index(idxu[:, 8 * r:8 * r + 8],
                                        vals[:, 8 * r:8 * r + 8], h4r[:])
                    nc.vector.match_replace(h4r[:],
                                            vals[:, 8 * r:8 * r + 8],
                                            h4r[:], -1e30)
                nc.sync.dma_start(
                    valbuf_d.ap().rearrange("(g k) -> g k", g=GPC), vals[:])
                nc.sync.dma_start(valrow[:],
                                  valbuf_d.ap().rearrange("(p e) -> p e", p=1))
                nc.vector.tensor_tensor(out=idxu[:], in0=idxu[:],
                                        in1=goff[:], op=OP.add)
                nc.sync.dma_start(
                    idxbuf_d.ap().rearrange("(g k) -> g k", g=GPC),
                    idxu[:].bitcast(mybir.dt.int16))
                idxw = wp.tile([96, 128], mybir.dt.int16, tag="idxw")
                nc.sync.dma_start(
                    idxw[0:16, :],
                    idxbuf_d.ap().rearrange("(c p) -> p c", p=16))
                for rep in range(1, 6):
                    nc.sync.dma_start(idxw[16 * rep:16 * rep + 16, :],
                                      idxw[0:16, :])
                nc.gpsimd.ap_gather(topsT[:], hcat16[:], idxw[:],
                                    channels=96, num_elems=256 * GPC,
                                    d=2, num_idxs=32 * GPC)

            # ---- head: conv1(97->16) -> maxpool2 -> conv2(16->32,k=5)
            #      -> fc 352->128 -> fc 128->1 ----
            with (
                tc.tile_pool(name="head", bufs=2) as hp,
                tc.tile_pool(name="psHd", bufs=1, space="PSUM") as psHd,
            ):
                c1T = hp.tile([16, 30 * GPC], bf16, tag="c1T")
                for q in range(4):
                    pc1 = psHd.tile([16, 480], fp32, tag="c1", bufs=2,
                                    name="pc1")
                    rhs = _apf(topsT[0:96, 1024 * q:1024 * q + 1],
                               [[64, 16], [2, 30]])
                    nc.tensor.matmul(pc1[:], lhsT=cw1[:], rhs=rhs,
                                     start=True, stop=False)
                    rhsv = _apf(valrow[0:1, 512 * q:512 * q + 1],
                                [[32, 16], [1, 30]])
                    nc.tensor.matmul(pc1[:], lhsT=cw1b[:], rhs=rhsv,
                                     start=False, stop=True)
                    nc.scalar.activation(c1T[:, 480 * q:480 * q + 480],
                                         pc1[:], AF.Relu, bias=cb1[:])
                poolT = hp.tile([16, 15 * GPC], bf16, tag="poolT")
                nc.vector.tensor_tensor(
                    out=_apf(poolT[0:16, 0:1], [[15, GPC], [1, 15]]),
                    in0=_apf(c1T[0:16, 0:1], [[30, GPC], [2, 15]]),
                    in1=_apf(c1T[0:16, 1:2], [[30, GPC], [2, 15]]),
                    op=OP.max)
                c2T = hp.tile([32, 11 * GPC], bf16, tag="c2T")
                for q in range(2):
                    pc2 = psHd.tile([32, 352], fp32, tag="c2", bufs=2,
                                    name="pc2")
                    for t in range(5):
                        rhs = _apf(poolT[0:16, 480 * q + t:480 * q + t + 1],
                                   [[15, 32], [1, 11]])
                        nc.tensor.matmul(pc2[:], lhsT=cw2[t][:], rhs=rhs,
                                         start=(t == 0), stop=(t == 4))
                    nc.scalar.activation(c2T[:, 352 * q:352 * q + 352],
                                         pc2[:], AF.Relu, bias=cb2[:])
                # flat[g, o*11+p]: 11 transposes of [32,64] slices
                c2n = hp.tile([64, 352], bf16, tag="c2n")
                for p in range(11):
                    pt = psHd.tile([64, 32], bf16, tag="pT", name="pt")
                    nc.tensor.transpose(pt[:],
                                        _apf(c2T[0:32, p:p + 1], [[11, GPC]]),
                                        ident16[0:32, 0:32])
                    nc.vector.tensor_copy(
                        _apf(c2n[0:64, p:p + 1], [[11, 32]]), pt[:])
                ft = [hp.tile([128, 64], bf16, tag=f"ft{q}", name=f"ft{q}")
                      for q in range(3)]
                for q in range(3):
                    w = 128 if q < 2 else 96
                    pf = psHd.tile([128, 64], bf16, tag="fT", name="pf")
                    nc.tensor.transpose(pf[0:w, :],
                                        c2n[:, 128 * q:128 * q + w],
                                        ident16[:])
                    nc.vector.tensor_copy(ft[q][0:w, :], pf[0:w, :])
                ph = psHd.tile([128, 64], fp32, tag="hl")
                for q in range(3):
                    w = 128 if q < 2 else 96
                    nc.tensor.matmul(ph[:], lhsT=lw1[q][0:w, :],
                                     rhs=ft[q][0:w, :],
                                     start=(q == 0), stop=(q == 2))
                hlinT = hp.tile([128, 64], bf16, tag="hlinT")
                nc.scalar.activation(hlinT[:], ph[:], AF.Relu, bias=lb1[:])
                po = psHd.tile([1, 64], fp32, tag="po")
                nc.tensor.matmul(po[:], lhsT=lw2[:], rhs=hlinT[:],
                                 start=True, stop=True)
                outT = hp.tile([1, 64], fp32, tag="outT")
                nc.scalar.activation(outT[:], po[:], AF.Sigmoid, bias=lb2[:])
                nc.sync.dma_start(out_d.ap(), outT[:])

    nc.compile()
    return nc


def _prep_inputs(inputs):
    """Shard + densify on host (integer graph structure only)."""
    bf = ml_dtypes.bfloat16
    x = np.asarray(inputs["x"], np.float32)
    ei = np.asarray(inputs["edge_index"], np.int64)
    src, dst = ei[0], ei[1]
    g_edge = dst // M
    jl = src - g_edge * M
    il = dst - g_edge * M
    flat = g_edge * (M * M) + jl * M + il
    cnt = np.bincount(flat, minlength=B * M * M).astype(np.float32)
    adj = cnt.reshape(B, M, M)
    adj += np.eye(M, dtype=np.float32)[None]

    # pack counts: [B, 128, 400] (j-lo cols 0:200, j-hi cols 200:400)
    cntp = np.zeros((B, 128, 400), np.float32)
    cntp[:, :, 0:200] = adj[:, 0:128, :]
    cntp[:, 0:72, 200:400] = adj[:, 128:200, :]
    cntp = cntp.astype(bf)

    # pack x: [B, 128, 256] (node-lo cols 0:128, node-hi cols 128:256)
    xr = x.reshape(B, M, 128)
    xp = np.zeros((B, 128, 256), np.float32)
    xp[:, :, 0:128] = xr[:, 0:128, :]
    xp[:, 0:72, 128:256] = xr[:, 128:200, :]
    xp = xp.astype(bf)

    degr = adj.sum(axis=1)  # [B, 200] integer-valued fp32 in-degree+1

    w234 = np.concatenate(
        [np.asarray(inputs["W2"], np.float32),
         np.asarray(inputs["W3"], np.float32),
         np.pad(np.asarray(inputs["W4"], np.float32), ((0, 0), (0, 31)))],
        axis=0)
    b4p = np.pad(np.asarray(inputs["b4"], np.float32), (0, 31))
    bgcn = np.stack(
        [np.asarray(inputs["b1"], np.float32),
         np.asarray(inputs["b2"], np.float32),
         np.asarray(inputs["b3"], np.float32), b4p], axis=1)
    cw1 = np.ascontiguousarray(
        np.asarray(inputs["convW1"], np.float32)[:, 0, :].T)
    cw2_r = np.asarray(inputs["convW2"], np.float32)
    cw2 = np.ascontiguousarray(cw2_r.transpose(2, 1, 0).reshape(80, 32))
    common = {
        "w1": np.asarray(inputs["W1"], np.float32).astype(bf),
        "w234": np.ascontiguousarray(w234).astype(bf),
        "bgcn": np.ascontiguousarray(bgcn),
        "cw1": cw1.astype(bf),
        "cb1": np.asarray(inputs["convb1"], np.float32).reshape(16, 1),
        "cw2": cw2.astype(bf),
        "cb2": np.asarray(inputs["convb2"], np.float32).reshape(32, 1),
        "lw1": np.asarray(inputs["linW1"], np.float32).astype(bf),
        "lb1": np.asarray(inputs["linb1"], np.float32).reshape(128, 1),
        "lw2": np.asarray(inputs["linW2"], np.float32).astype(bf),
        "lb2": np.asarray(inputs["linb2"], np.float32).reshape(1, 1),
    }
    in_maps = []
    for c in range(NCORES):
        m = dict(common)
        m["cnt"] = np.ascontiguousarray(
            cntp[GPC * c:GPC * (c + 1)].reshape(GPC * 128, 400))
        m["x"] = np.ascontiguousarray(
            xp[GPC * c:GPC * (c + 1)].reshape(GPC * 128, 256))
        m["degr"] = np.ascontiguousarray(degr[GPC * c:GPC * (c + 1)])
        in_maps.append(m)
    return in_maps


def _run(inputs, trace=False):
    from concourse import bass_utils
    if "nc" not in _STATE:
        _STATE["nc"] = _build()
    nc = _STATE["nc"]
    in_maps = _prep_inputs(inputs)
    res = bass_utils.run_bass_kernel_spmd(
        nc, in_maps, core_ids=list(range(NCORES)), trace=trace)
    out = np.concatenate([res.results[c]["out"].reshape(GPC)
                          for c in range(NCORES)])
    return out.reshape(B, 1).astype(np.float32), res


def kernel(**inputs) -> np.ndarray:
    out, _ = _run(inputs, trace=False)
    return out
